# revision 22
# baseline (speedup 1.0000x reference)
"""AttentionBlock (GroupNorm + single-head full attention + residual) on 8
Trainium2 NeuronCores.

Sharding: data-parallel over batch (4) x sequence-parallel over query
tokens (2 halves of h*w=4096). Each core gets its batch slice with the
token axis ROTATED by the host so that its 2048 queries are always
columns 0:NQ (attention is permutation-invariant over keys, GroupNorm
over positions), so a single xb input serves stats, K, V and the query
slice. No collectives; the host scatters inputs and gathers outputs.

Per-core pipeline (channels on partitions; projections in float32r,
attention in fp8e4 DoubleRow = 2 rows/PE-cell, c=256 contraction in one
128-partition matmul):
 - GroupNorm stats per 2048-col chunk as DMAs land (DVE sum, ACT
   Square+accum); group reduce + broadcast via tiny indicator matmuls.
   Normalization is FOLDED INTO THE WEIGHTS (W' = W*a per in-channel,
   bias' = W@b + bias) so all matmuls consume RAW x.
 - out_w is folded into V (W2 = out_w @ Wv on device); Q/K/out_w arrive
   pre-transposed from the host.
 - Q, K are written by the projection bias-copies directly as fp8e4 in
   DoubleRow layout [128, 2(c-half), n]; V2^T likewise as fp8 [128,
   32(k-tile), 256].
 - Attention per 512-query chunk: one DR matmul per k-tile gives
   S^T pairs [128, 1024] in PSUM (2 banks); ONE exp per pair on ACT
   (scale 1/16, bias -3) emits P^T as fp8 [128, 2, 512]. PV runs
   TRANSPOSED (stationary = V2^T k-pair slice, reused; moving = P^T,
   DR) accumulating O'^T[c,q] over 16 pairs; a third DR matmul with a
   ones stationary accumulates the softmax denominator l[q] (the e^-3
   bias cancels in O'/l). 1/l is PE-broadcast to 128 partitions and
   applied with DVE mult+add into the residual y - no output transposes.
 - PSUM: proj uses a 3x[128,1024] ring; attention re-slices the space
   into 2x[128,1024] S^T pairs + 2 O'^T banks + l + 1/l-broadcast.

Toolchain notes: walrus accepts one sync-wait per instruction
(SplitWaitTileContext splits the rest onto nops); f32r consumers need
f32r-rounding producers (gpsimd casting DMAs); fp8 DR needs 3D APs
[Ki, 2, dim] with middle step % 16 == 0; PSUM pools are time-sliced via
nested ExitStacks (proj ring closed before attention pools open).
"""

import numpy as np

B, C, HW = 4, 256, 4096
NQ = HW // 2
G = 8
CPG = C // G  # channels per group
EPS = 1e-5
N_CORES = 8
USE_FP8_PV = False
EXP_BIAS = -3.0

_CACHE = {}


def _build_nc(loop_reps=1, debug=False):
    import bass_rust
    import concourse.bass as bass
    import concourse.mybir as mybir
    import concourse.tile as tile
    from concourse.masks import make_identity
    from concourse.vector_clock import ScopedClock

    F32 = mybir.dt.float32
    FR = mybir.dt.float32r
    F8 = mybir.dt.float8e4
    BF = mybir.dt.bfloat16
    AF = mybir.ActivationFunctionType
    ALU = mybir.AluOpType

    MAXW = 1

    class SplitWaitTileContext(tile.TileContext):
        """Workaround: this toolchain's walrus accepts at most one sync-wait
        per instruction; split excess waits onto same-engine InstNoOps."""

        def _split_excess_waits(self, inst):
            si = inst.sync_info
            if si is None:
                return []
            waits = list(si.on_wait)
            if len(waits) <= MAXW:
                return []
            extra, keep = waits[:-MAXW], waits[-MAXW:]
            nops = [
                mybir.InstNoOp(
                    name=f"I-{self.nc.next_id()}",
                    sync_info=mybir.SyncInfo(on_wait=[w], on_update=[]),
                    bass_nofuse=True,
                    engine=inst.engine,
                )
                for w in extra
            ]
            inst.sync_info = mybir.SyncInfo(on_wait=keep, on_update=list(si.on_update))
            return nops

        def _commit_and_lower(self, inst, original_block, old_bb_map, bb_to_exit_bb):
            for nop in self._split_excess_waits(inst):
                self._commit_instruction(nop, lazy_reg_writes=False)
            return super()._commit_and_lower(
                inst, original_block, old_bb_map, bb_to_exit_bb
            )

        def _drain_and_barrier(self, tick_clock, wait_clock):
            drain_inst = self.nc.sync.drain()
            wait_clock.add_sem_waits(
                drain_inst.ins, ScopedClock({None: tick_clock.global_clock})
            )
            si = drain_inst.ins.sync_info
            waits = list(si.on_wait) if si is not None else []
            if len(waits) > MAXW:
                updates = list(si.on_update) if si is not None else []
                drain_inst.ins.sync_info = bass_rust.SyncInfo(
                    on_wait=waits[:MAXW], on_update=[]
                )
                rest = waits[MAXW:]
                for i, w in enumerate(rest):
                    extra = self.nc.sync.drain()
                    extra.ins.sync_info = bass_rust.SyncInfo(
                        on_wait=[w], on_update=updates if i == len(rest) - 1 else []
                    )
            self.nc.all_engine_barrier()
            assert self.sems is not None
            popped = self.nc._tile_sem_poison_stack.pop()
            assert popped is self._sem_poison
            self.nc.clear_and_free_semaphores(list(self.sems.allocated().values()))
            self.nc.all_engine_barrier()

    nc = bass.Bass()
    xb = nc.dram_tensor("xb", [C, HW], F32, kind="ExternalInput")
    qkv_w = nc.dram_tensor("qkv_w", [3 * C, C], F32, kind="ExternalInput")
    qkv_b = nc.dram_tensor("qkv_b", [3 * C], F32, kind="ExternalInput")
    out_w = nc.dram_tensor("out_w", [C, C], F32, kind="ExternalInput")
    out_b = nc.dram_tensor("out_b", [C], F32, kind="ExternalInput")
    gn_gamma = nc.dram_tensor("gn_gamma", [C], F32, kind="ExternalInput")
    gn_beta = nc.dram_tensor("gn_beta", [C], F32, kind="ExternalInput")
    gind_in = nc.dram_tensor("gind_in", [128, 16], F32, kind="ExternalInput")
    hind_in = nc.dram_tensor("hind_in", [8, 128 * 2], F32, kind="ExternalInput")
    ident_in = nc.dram_tensor("ident_in", [128, 128], F32, kind="ExternalInput")
    wqkT_in = nc.dram_tensor("wqkT_in", [C, 512], F32, kind="ExternalInput")
    owT_in = nc.dram_tensor("owT_in", [C, C], F32, kind="ExternalInput")
    y = nc.dram_tensor("y", [C, NQ], F32, kind="ExternalOutput")
    if debug:
        d_xn = nc.dram_tensor("d_xn", [C, HW], F32, kind="ExternalOutput")
        d_q = nc.dram_tensor("d_q", [C, NQ], F32, kind="ExternalOutput")
        d_k = nc.dram_tensor("d_k", [C, HW], F32, kind="ExternalOutput")
        d_v2t = nc.dram_tensor("d_v2t", [HW, 272], F32, kind="ExternalOutput")
        d_po = nc.dram_tensor("d_po", [128, 272], F32, kind="ExternalOutput")
        d_ab = nc.dram_tensor("d_ab", [C, 2], F32, kind="ExternalOutput")

    with SplitWaitTileContext(nc) as tc:
        import contextlib

        ctx = contextlib.ExitStack()
        with ctx:
            singles = ctx.enter_context(tc.tile_pool(name="singles", bufs=1))
            xpool = ctx.enter_context(tc.tile_pool(name="xpool", bufs=2))
            qpool = ctx.enter_context(tc.tile_pool(name="qpool", bufs=2))
            kpool = ctx.enter_context(tc.tile_pool(name="kpool", bufs=2))
            vpool = ctx.enter_context(tc.tile_pool(name="vpool", bufs=2))
            ypool = ctx.enter_context(tc.tile_pool(name="ypool", bufs=2))
            wpool = ctx.enter_context(tc.tile_pool(name="wpool", bufs=1))
            wnat = ctx.enter_context(tc.tile_pool(name="wnat", bufs=1))
            ppool = ctx.enter_context(tc.tile_pool(name="ppool", bufs=3))
            opool = ctx.enter_context(tc.tile_pool(name="opool", bufs=3))
            small = ctx.enter_context(tc.tile_pool(name="small", bufs=4))
            stat = ctx.enter_context(tc.tile_pool(name="stat", bufs=2))
            scpool = ctx.enter_context(tc.tile_pool(name="scpool", bufs=2))

            def setup():
                # constants: allocate tiles; DMAs deferred so weight/x loads
                # hit the serial DMA-issue paths first.
                ident = singles.tile([128, 128], FR, tag="ident")
                eps_sb = singles.tile([8, 1], F32, tag="eps")
                nc.vector.memset(eps_sb, EPS)
                ebias_sb = singles.tile([128, 1], F32, tag="ebias")
                nc.vector.memset(ebias_sb, EXP_BIAS)
                gam_sb = singles.tile([128, 2], F32, tag="gam")
                bet_sb = singles.tile([128, 2], F32, tag="bet")
                qb_sb = singles.tile([128, 6], F32, tag="qb")
                ob_sb = singles.tile([128, 2], F32, tag="ob")
                gi_sb = singles.tile([128, 16], F32, tag="gi")
                hi_sb = singles.tile([8, 128 * 2], F32, tag="hi")

                def load_consts():
                    nc.sync.dma_start(out=qb_sb, in_=qkv_b.rearrange("(m p) -> p m", p=128))
                    nc.sync.dma_start(out=gam_sb, in_=gn_gamma.rearrange("(t p) -> p t", p=128))
                    nc.sync.dma_start(out=bet_sb, in_=gn_beta.rearrange("(t p) -> p t", p=128))
                    nc.sync.dma_start(out=ob_sb, in_=out_b.rearrange("(t p) -> p t", p=128))
                    nc.sync.dma_start(out=gi_sb, in_=gind_in[:, :])
                    nc.sync.dma_start(out=hi_sb, in_=hind_in[:, :])

                g_sb = [gam_sb[:, t : t + 1] for t in range(2)]
                be_sb = [bet_sb[:, t : t + 1] for t in range(2)]
                gind = [gi_sb[:, 0:8], gi_sb[:, 8:16]]
                hind = [hi_sb[:, 0:128], hi_sb[:, 128:256]]
                return (ident, g_sb, be_sb, qb_sb, ob_sb, eps_sb, ebias_sb, gind, hind, load_consts)

            def body(rep, consts):
                (ident, g_sb, be_sb, qb_sb, ob_sb, eps_sb, ebias_sb, gind, hind, load_consts) = consts
                # PSUM is time-sliced: proj phase gets a 2x[128,1024] ring
                # (4 banks); attention re-uses the space for S^T pair tiles
                # plus the 4 po accumulators.
                pctx = contextlib.ExitStack()
                psmm = pctx.enter_context(
                    tc.tile_pool(name="psproj", bufs=3, space="PSUM")
                )
                # x first on the gpsimd/transfer path (stats-critical)
                # f32r targets must use gpsimd casting DMAs (BIR verifier
                # rejects other producers feeding f32r matmuls). Coarsen x to
                # 2048-col chunks, interleaved across halves, to cut SWDGE
                # descriptor-gen serialization on the pool queue.
                x_sb = [
                    xpool.tile([128, HW], FR, tag="xv", name=f"x{t}")
                    for t in range(2)
                ]
                for c2 in range(2):
                    for t in range(2):
                        nc.gpsimd.dma_start(
                            out=x_sb[t][:, c2 * 2048 : (c2 + 1) * 2048],
                            in_=xb[t * 128 : (t + 1) * 128, c2 * 2048 : (c2 + 1) * 2048],
                        )
                # host pre-rotates xb per core so the query half is always
                # columns 0:NQ (attention is permutation-invariant over keys)
                xq_sb = [x_sb[t][:, 0:NQ] for t in range(2)]

                # weights arrive pre-transposed from the host
                wT = []  # (Wq|Wk)^T tiles [c_in 128, 512] f32r
                for t in range(2):
                    wT.append(wpool.tile([128, 512], FR, tag=f"wT{t}", name=f"wTn{t}"))
                owT = []  # out_w^T tiles [c_in 128, 256] f32r
                for t in range(2):
                    owT.append(wpool.tile([128, 256], FR, tag=f"owT{t}", name=f"owT{t}"))
                for t in range(2):
                    nc.gpsimd.dma_start(out=owT[t], in_=owT_in[t * 128 : (t + 1) * 128, :])
                for t in range(2):
                    nc.gpsimd.dma_start(out=wT[t], in_=wqkT_in[t * 128 : (t + 1) * 128, :])
                load_consts()

                # ---------- W2 = out_w @ Wv fold ----------
                wv_fr = []
                for i in range(2):
                    wv = wpool.tile([128, C], FR, tag=f"wv{i}", name=f"wv{i}")
                    nc.gpsimd.dma_start(
                        out=wv, in_=qkv_w[512 + i * 128 : 512 + (i + 1) * 128, :]
                    )
                    wv_fr.append(wv)
                w2t = []
                for t in range(2):
                    ps = psmm.tile([128, 256], F32, tag="mm")
                    nc.tensor.matmul(
                        ps, wv_fr[0][:, t * 128 : (t + 1) * 128], owT[0],
                        start=True, stop=False,
                    )
                    nc.tensor.matmul(
                        ps, wv_fr[1][:, t * 128 : (t + 1) * 128], owT[1],
                        start=False, stop=True,
                    )
                    w2 = wpool.tile([128, 256], FR, tag=f"w2t{t}", name=f"w2t{t}")
                    nc.vector.tensor_copy(w2, ps)
                    w2t.append(w2)
                # ob_eff = out_b + out_w @ bv  (bv = qkv_b[512:768])
                ps_ob = psmm.tile([128, 2], F32, tag="mm")
                for m2 in range(2):
                    nc.tensor.matmul(
                        ps_ob[:, m2 : m2 + 1],
                        owT[0][:, m2 * 128 : (m2 + 1) * 128].bitcast(F32),
                        qb_sb[:, 4:5],
                        start=True, stop=False,
                    )
                    nc.tensor.matmul(
                        ps_ob[:, m2 : m2 + 1],
                        owT[1][:, m2 * 128 : (m2 + 1) * 128].bitcast(F32),
                        qb_sb[:, 5:6],
                        start=False, stop=True,
                    )
                ob_eff = stat.tile([128, 2], F32, tag="obeff")
                nc.vector.tensor_add(ob_eff, ps_ob, ob_sb)


                # ---------- GroupNorm stats ----------
                # per-channel raw sums: DVE does sum(x) while ACT does
                # sum(x^2) via Square+accum_out (parallel engines).
                st2 = []
                for t in range(2):
                    s1m = stat.tile([128, 4], F32, tag=f"s1m{t}", name=f"s1m{t}")
                    s2m = stat.tile([128, 4], F32, tag=f"s2m{t}", name=f"s2m{t}")
                    for cck in range(4):
                        sl = slice(cck * 1024, (cck + 1) * 1024)
                        nc.vector.reduce_sum(
                            out=s1m[:, cck : cck + 1],
                            in_=x_sb[t][:, sl].bitcast(F32),
                            axis=mybir.AxisListType.X,
                        )
                        sq = scpool.tile([128, 1024], F32, tag="sc", name=f"sq{t}{cck}")
                        nc.scalar.activation(
                            out=sq, in_=x_sb[t][:, sl].bitcast(F32),
                            func=AF.Square, accum_out=s2m[:, cck : cck + 1],
                        )
                    s2t = stat.tile([128, 2], F32, tag=f"st2{t}")
                    nc.vector.reduce_sum(out=s2t[:, 0:1], in_=s1m, axis=mybir.AxisListType.X)
                    nc.vector.reduce_sum(out=s2t[:, 1:2], in_=s2m, axis=mybir.AxisListType.X)
                    st2.append(s2t)
                psg = psmm.tile([8, 2], F32, tag="mm")
                nc.tensor.matmul(psg, gind[0], st2[0], start=True, stop=False)
                nc.tensor.matmul(psg, gind[1], st2[1], start=False, stop=True)
                gstat = stat.tile([8, 2], F32, tag="gstat")  # [mean_g, E[x^2]_g]
                nc.vector.tensor_scalar_mul(gstat, psg, 1.0 / (CPG * HW))
                var_g = stat.tile([8, 1], F32, tag="varg")
                nc.vector.tensor_mul(var_g, gstat[:, 0:1], gstat[:, 0:1])
                nc.vector.tensor_sub(var_g, gstat[:, 1:2], var_g)
                std_g = stat.tile([8, 1], F32, tag="stdg")
                nc.scalar.activation(out=std_g, in_=var_g, func=AF.Sqrt, bias=eps_sb, scale=1.0)
                rm = stat.tile([8, 2], F32, tag="rm")  # [rstd_g, mean_g]
                nc.vector.reciprocal(rm[:, 0:1], std_g)
                nc.vector.tensor_copy(rm[:, 1:2], gstat[:, 0:1])
                # broadcast to channels: [rstd_c, mean_c] = H_t.T @ rm
                ab = []
                for t in range(2):
                    psb = psmm.tile([128, 2], F32, tag="mm")
                    nc.tensor.matmul(psb, hind[t], rm, start=True, stop=True)
                    abt = stat.tile([128, 2], F32, tag=f"ab{t}")  # [a_c, b_c]
                    nc.vector.tensor_mul(abt[:, 0:1], psb[:, 0:1], g_sb[t])
                    nc.vector.tensor_mul(abt[:, 1:2], psb[:, 1:2], abt[:, 0:1])
                    nc.vector.tensor_sub(abt[:, 1:2], be_sb[t], abt[:, 1:2])
                    ab.append(abt)

                # ---------- fold GN into weights: no x_norm pass ----------
                # K/Q/V2 consume RAW x; W' = W * a (per c_in), biases get W@b.
                # Bias matmuls (plain fp32, N=1-2) use the UNSCALED weights;
                # the in-place scales below are WAR-ordered after them.
                wTs = []
                w2ts = []
                for t in range(2):
                    wt2 = wpool.tile([128, 512], FR, tag=f"wTs{t}", name=f"wTs{t}")
                    nc.vector.tensor_scalar_mul(wt2, wT[t], ab[t][:, 0:1])
                    wTs.append(wt2)
                    w22 = wpool.tile([128, 256], FR, tag=f"w2ts{t}", name=f"w2ts{t}")
                    nc.vector.tensor_scalar_mul(w22, w2t[t], ab[t][:, 0:1])
                    w2ts.append(w22)
                ps_qb = psmm.tile([128, 4], F32, tag="mm", name="ps_qb")
                for m in range(4):
                    nc.tensor.matmul(
                        ps_qb[:, m : m + 1],
                        wT[0][:, m * 128 : (m + 1) * 128].bitcast(F32),
                        ab[0][:, 1:2],
                        start=True, stop=False,
                    )
                    nc.tensor.matmul(
                        ps_qb[:, m : m + 1],
                        wT[1][:, m * 128 : (m + 1) * 128].bitcast(F32),
                        ab[1][:, 1:2],
                        start=False, stop=True,
                    )
                qb_eff = stat.tile([128, 4], F32, tag="qbeff")
                nc.vector.tensor_add(qb_eff, ps_qb, qb_sb[:, 0:4])
                ps_ob2 = psmm.tile([128, 2], F32, tag="mm", name="ps_ob2")
                for m2 in range(2):
                    nc.tensor.matmul(
                        ps_ob2[:, m2 : m2 + 1],
                        w2t[0][:, m2 * 128 : (m2 + 1) * 128].bitcast(F32),
                        ab[0][:, 1:2],
                        start=True, stop=False,
                    )
                    nc.tensor.matmul(
                        ps_ob2[:, m2 : m2 + 1],
                        w2t[1][:, m2 * 128 : (m2 + 1) * 128].bitcast(F32),
                        ab[1][:, 1:2],
                        start=False, stop=True,
                    )
                ob_f = stat.tile([128, 2], F32, tag="obf")
                nc.vector.tensor_add(ob_f, ps_ob2, ob_eff)

                # residual prep from raw xq bits
                y_sb = []
                for t in range(2):
                    yt = ypool.tile([128, NQ], F32, tag="y", name=f"y{t}")
                    nc.vector.tensor_scalar_add(
                        yt, xq_sb[t][:, :].bitcast(F32), ob_f[:, t : t + 1]
                    )
                    y_sb.append(yt)
                xn = x_sb
                xqn = xq_sb

                # ---------- qkv projections ----------
                # Q and K are written as fp8e4 in DoubleRow layout
                # [128, 2, n] (c-halves packed in the middle dim) so the
                # scores matmul runs one fp8 DR matmul per (kt, qc).
                q8 = qpool.tile([128, 2, NQ], F8, tag="q", name="q8")
                k8 = kpool.tile([128, 2, HW], F8, tag="k", name="k8")
                nch = 0
                for m in (2, 3, 0, 1):
                    dst = q8 if m < 2 else k8
                    src = xqn if m < 2 else xn
                    nj = (NQ if m < 2 else HW) // 1024
                    for j in range(nj):
                        ps = psmm.tile([128, 1024], F32, tag="mm")
                        for i in range(2):
                            sl = slice((2 * j + i) * 512, (2 * j + i + 1) * 512)
                            nc.tensor.matmul(
                                ps[:, i * 512 : (i + 1) * 512],
                                wTs[0][:, m * 128 : (m + 1) * 128],
                                src[0][:, sl],
                                start=True,
                                stop=False,
                            )
                            nc.tensor.matmul(
                                ps[:, i * 512 : (i + 1) * 512],
                                wTs[1][:, m * 128 : (m + 1) * 128],
                                src[1][:, sl],
                                start=False,
                                stop=True,
                            )
                        dslice = dst[:, m % 2, j * 1024 : (j + 1) * 1024]
                        if nch % 2 == 0:
                            nc.vector.tensor_scalar_add(dslice, ps, qb_eff[:, m : m + 1])
                        else:
                            nc.scalar.activation(
                                out=dslice, in_=ps, func=AF.Identity,
                                bias=qb_eff[:, m : m + 1], scale=1.0,
                            )
                        nch += 1

                # ---------- V2^T = xn^T @ W2^T, fp8 [k-tile, c] ----------
                # v28[p, nt, c] = V2^T[nt*128+p, c]; PV runs transposed
                # (stationary = V2T k-pair slices, moving = P^T) so the
                # stationary is reused and fp8 DoubleRow halves the stream.
                v28 = vpool.tile([128, 32, 256], F8, tag="v2", name="v28")
                for ntq in range(8):
                    ps = psmm.tile([128, 1024], F32, tag="mm")
                    for i in range(4):
                        nt = 4 * ntq + i
                        nc.tensor.matmul(
                            ps[:, i * 256 : (i + 1) * 256],
                            xn[0][:, nt * 128 : (nt + 1) * 128], w2ts[0],
                            start=True, stop=False,
                        )
                        nc.tensor.matmul(
                            ps[:, i * 256 : (i + 1) * 256],
                            xn[1][:, nt * 128 : (nt + 1) * 128], w2ts[1],
                            start=False, stop=True,
                        )
                    dst = v28[:, 4 * ntq : 4 * ntq + 4, :]
                    if ntq % 2 == 0:
                        nc.vector.tensor_copy(dst, ps)
                    else:
                        nc.scalar.copy(dst, ps)
                # fp8 DR stationary needs middle-dim step % 16 == 0 and
                # memset can't write fp8: build the ones stationary as
                # [128, 2, 16] via an ACT copy from an f32 ones tile.
                ones_f = singles.tile([128, 32], F32, tag="onesf")
                nc.vector.memset(ones_f, 1.0)
                ones8 = singles.tile([128, 2, 16], F8, tag="ones8")
                nc.scalar.copy(ones8, ones_f)
                ones1f = singles.tile([1, 128], F32, tag="ones1f")
                nc.vector.memset(ones1f, 1.0)
                ones1 = singles.tile([1, 128], FR, tag="ones1")
                nc.vector.tensor_copy(ones1, ones1f)

                if debug:
                    for t in range(2):
                        nc.sync.dma_start(
                            out=d_xn[t * 128 : (t + 1) * 128, :],
                            in_=xn[t][:, :].bitcast(F32),
                        )
                        nc.sync.dma_start(
                            out=d_ab[t * 128 : (t + 1) * 128, :], in_=ab[t]
                        )

                # ---------- attention ----------
                pctx.close()
                actx = contextlib.ExitStack()
                psattn = actx.enter_context(
                    tc.tile_pool(name="psattn", bufs=2, space="PSUM")
                )
                pspo = actx.enter_context(
                    tc.tile_pool(name="pspo", bufs=2, space="PSUM")
                )
                pspl = actx.enter_context(
                    tc.tile_pool(name="pspl", bufs=1, space="PSUM")
                )
                psrb = actx.enter_context(
                    tc.tile_pool(name="psrb", bufs=1, space="PSUM")
                )
                for qc in range(NQ // 512):
                    # poT[ch] accumulates O'^T[c,q] (un-normalized, x e^-3);
                    # pol row 0 accumulates the softmax denominator l (x e^-3).
                    poT = [
                        pspo.tile([128, 512], F32, tag="o", name=f"poT{ch}")
                        for ch in range(2)
                    ]
                    pol = pspl.tile([2, 512], F32, tag="l", name="pol")
                    for jp in range(16):
                        psp = psattn.tile([128, 1024], F32, tag="pair")
                        for i in range(2):
                            kt = 2 * jp + i
                            nc.tensor.matmul(
                                psp[:, i * 512 : (i + 1) * 512],
                                k8[:, :, kt * 128 : (kt + 1) * 128],
                                q8[:, :, qc * 512 : (qc + 1) * 512],
                                start=True, stop=True,
                                perf_mode=mybir.MatmulPerfMode.DoubleRow,
                            )
                        pT8 = ppool.tile([128, 2, 512], F8, tag="p")
                        nc.scalar.activation(
                            out=pT8, in_=psp, func=AF.Exp, scale=1.0 / 16.0,
                            bias=ebias_sb,
                        )
                        for ch in range(2):
                            nc.tensor.matmul(
                                poT[ch],
                                v28[:, 2 * jp : 2 * jp + 2, ch * 128 : (ch + 1) * 128],
                                pT8,
                                start=(jp == 0), stop=(jp == 15),
                                perf_mode=mybir.MatmulPerfMode.DoubleRow,
                                skip_group_check=True,
                            )
                        nc.tensor.matmul(
                            pol,
                            ones8[:, :, 0:2],
                            pT8,
                            start=(jp == 0), stop=(jp == 15),
                            perf_mode=mybir.MatmulPerfMode.DoubleRow,
                            skip_group_check=True,
                        )
                    # rl = 1/l broadcast to all 128 partitions via PE
                    rlv = small.tile([1, 512], FR, tag="rlv")
                    with nc.allow_low_precision(reason="rl bcast via f32r matmul"):
                        nc.vector.reciprocal(rlv, pol[0:1, :])
                    rlb = psrb.tile([128, 512], F32, tag="rlb", name="rlb")
                    nc.tensor.matmul(rlb, ones1, rlv, start=True, stop=True)
                    # engines may read only one PSUM operand per instruction
                    rlb_sb = opool.tile([128, 512], F32, tag="rlbs", name="rlb_sb")
                    nc.vector.tensor_copy(rlb_sb, rlb)
                    for t in range(2):
                        tmp = opool.tile([128, 512], F32, tag="tmp", name=f"tmp{t}")
                        nc.vector.tensor_tensor(tmp, poT[t], rlb_sb, ALU.mult)
                        ys = y_sb[t][:, qc * 512 : (qc + 1) * 512]
                        nc.vector.tensor_tensor(ys, tmp, ys, ALU.add)
                    for t in range(2):
                        nc.sync.dma_start(
                            out=y[t * 128 : (t + 1) * 128, qc * 512 : (qc + 1) * 512],
                            in_=y_sb[t][:, qc * 512 : (qc + 1) * 512],
                        )
                actx.close()

            consts = setup()
            for rep in range(loop_reps):
                body(rep, consts)

    return nc


def _get_runner(loop_reps=1):
    key = ("runner", loop_reps)
    if key not in _CACHE:
        nc = _build_nc(loop_reps)
        _CACHE[key] = nc
    return _CACHE[key]


K_USE_FP8 = USE_FP8_PV


def make_extra_inputs():
    gind = np.zeros((128, 16), dtype=np.float32)
    hind = np.zeros((8, 256), dtype=np.float32)
    for t in range(2):
        for p in range(128):
            g = (t * 128 + p) // CPG
            gind[p, t * 8 + g] = 1.0
            hind[g, t * 128 + p] = 1.0
    op = np.zeros((128, 256), dtype=np.float32)
    op[:, 0::16 if USE_FP8_PV else 2] = 1.0
    return {"gind_in": gind, "hind_in": hind, "ones_in": op,
            "ident_in": np.eye(128, dtype=np.float32)}


def make_weight_inputs(qkv_w, out_w):
    return {
        "wqkT_in": np.ascontiguousarray(qkv_w[0:512].T),
        "owT_in": np.ascontiguousarray(out_w.T),
    }


def kernel(x, gn_gamma, gn_beta, qkv_w, qkv_b, out_w, out_b):
    from concourse.bass_utils import run_bass_kernel_spmd

    x = np.asarray(x, dtype=np.float32)
    gn_gamma = np.asarray(gn_gamma, dtype=np.float32)
    gn_beta = np.asarray(gn_beta, dtype=np.float32)
    qkv_w = np.asarray(qkv_w, dtype=np.float32)
    qkv_b = np.asarray(qkv_b, dtype=np.float32)
    out_w = np.asarray(out_w, dtype=np.float32)
    out_b = np.asarray(out_b, dtype=np.float32)

    b, c, h, w = x.shape
    assert (b, c, h * w) == (B, C, HW)
    xf = x.reshape(b, c, HW)

    nc = _get_runner()
    in_maps = []
    for j in range(N_CORES):
        bi, qh = j // 2, j % 2
        if qh == 0:
            xbj = np.ascontiguousarray(xf[bi])
        else:
            xbj = np.concatenate([xf[bi][:, NQ:], xf[bi][:, :NQ]], axis=1)
        in_maps.append(
            {
                "xb": xbj,
                "qkv_w": qkv_w,
                "qkv_b": qkv_b,
                "out_w": out_w,
                "out_b": out_b,
                "gn_gamma": gn_gamma,
                "gn_beta": gn_beta,
            }
        )
    extras = make_extra_inputs()
    extras.update(make_weight_inputs(qkv_w, out_w))
    for m in in_maps:
        m.update(extras)
    res = run_bass_kernel_spmd(nc, in_maps, core_ids=list(range(N_CORES)))
    out = np.empty((B, C, HW), dtype=np.float32)
    for j in range(N_CORES):
        bi, qh = j // 2, j % 2
        out[bi][:, qh * NQ : (qh + 1) * NQ] = res.results[j]["y"]
    return out.reshape(b, c, h, w)

